# revision 1
# baseline (speedup 1.0000x reference)
"""GCN 2-layer encoder on 8 Trainium2 NeuronCores (Bass/Tile).

Math (PyG GCNConv x2, self-loops, symmetric norm):
    A' = A + I, deg = indegree(A'), dinv = deg^-1/2
    h1 = relu(dinv * (A' (dinv * (x W1))) + b1)
    out = dinv * (A' (dinv * (h1 W2))) + b2

Sharding: dst nodes split contiguously across 8 cores (12500 each). Each
core projects its own rows (x W), scales by dinv; an AllGather builds the
full projected table in DRAM; each core then aggregates its own dst rows
by gathering per-edge source rows (indirect DMA) and segment-summing with
DVE tensor_reduce over a degree-sorted, group-padded edge layout.

Host prep: nodes per core are sorted by degree and batched in groups of
128; group gather width D_g = max degree in the group (near-uniform after
sorting). The per-group widths are maxed across cores so all 8 cores run
one identical program (SPMD). Edge slots beyond a node's degree point at
an all-zeros table row.
"""

import numpy as np

N = 100000
IN_C, HID, OUT_C = 256, 32, 16
NCORES = 8
P = 128
NPC = N // NCORES            # nodes per core: 12500
NGROUP = (NPC + P - 1) // P  # 98 groups
NPAD = NGROUP * P            # 12544 rows per core slice (incl. dummies)
VTOT = NCORES * NPAD         # table rows: 100352


def _host_prep(x, edge_index, W1, b1, W2, b2):
    x = np.asarray(x, dtype=np.float32)
    ei = np.asarray(edge_index)
    W1 = np.asarray(W1, dtype=np.float32)
    b1 = np.asarray(b1, dtype=np.float32)
    W2 = np.asarray(W2, dtype=np.float32)
    b2 = np.asarray(b2, dtype=np.float32)

    loops = np.arange(N, dtype=np.int64)
    src = np.concatenate([ei[0], loops]).astype(np.int64)
    dst = np.concatenate([ei[1], loops]).astype(np.int64)

    deg = np.bincount(dst, minlength=N).astype(np.int64)
    dinv = (1.0 / np.sqrt(np.maximum(deg, 1))).astype(np.float32)

    core_of = (np.arange(N) // NPC).astype(np.int64)
    pos_in_core = np.empty(N, dtype=np.int64)
    perms = []
    for c in range(NCORES):
        nodes = np.arange(c * NPC, (c + 1) * NPC)
        perm = nodes[np.argsort(deg[nodes], kind="stable")]
        perms.append(perm)
        pos_in_core[perm] = np.arange(NPC)
    rowid = core_of * NPAD + pos_in_core  # table row of each node

    # per-(core, group) gather widths, maxed across cores for SPMD
    Dcg = np.zeros((NCORES, NGROUP), dtype=np.int64)
    for c in range(NCORES):
        dsort = deg[perms[c]]
        dpad = np.zeros(NPAD, dtype=np.int64)
        dpad[:NPC] = dsort
        Dcg[c] = dpad.reshape(NGROUP, P).max(axis=1)
    Dg = Dcg.max(axis=0)          # [NGROUP]
    Dg = np.maximum(Dg, 1)
    cumD = np.concatenate([[0], np.cumsum(Dg)]).astype(np.int64)
    sumD = int(cumD[-1])

    # CSR over table-row ids, then slot layout [core][p, cumD[g]+j]
    erow = rowid[dst]                              # dst slot row
    esrc_row = rowid[src].astype(np.int32)          # value to gather
    order = np.argsort(erow, kind="stable")
    erow_s = erow[order]
    esrc_s = esrc_row[order]
    counts = np.bincount(erow, minlength=VTOT)
    ptr = np.concatenate([[0], np.cumsum(counts)])
    j_idx = np.arange(erow_s.size, dtype=np.int64) - ptr[erow_s]

    c_arr = erow_s // NPAD
    within = erow_s % NPAD
    g_arr = within // P
    p_arr = within % P
    col_arr = cumD[g_arr] + j_idx

    zero_row = np.array([c * NPAD + NPC for c in range(NCORES)], dtype=np.int32)
    offs = np.empty((NCORES, P, sumD), dtype=np.int32)
    for c in range(NCORES):
        offs[c, :, :] = zero_row[c]
    offs[c_arr, p_arr, col_arr] = esrc_s

    # per-core inputs
    xT_list, dinv_list = [], []
    for c in range(NCORES):
        xp = np.zeros((NPAD, IN_C), dtype=np.float32)
        xp[:NPC] = x[perms[c]]
        xT_list.append(np.ascontiguousarray(xp.T))
        dv = np.zeros(NPAD, dtype=np.float32)
        dv[:NPC] = dinv[perms[c]]
        dinv_list.append(np.ascontiguousarray(
            dv.reshape(NGROUP, P).T))  # [128, NGROUP]

    b1b = np.tile(b1[None, :], (P, 1)).astype(np.float32)
    b2b = np.tile(b2[None, :], (P, 1)).astype(np.float32)

    return dict(
        Dg=Dg, cumD=cumD, sumD=sumD, offs=offs, xT=xT_list, dinv=dinv_list,
        W1=W1, W2=W2, b1b=b1b, b2b=b2b, perms=perms,
    )


_NC_CACHE = {}


def _build_bass(Dg, sumD):
    key = (tuple(int(d) for d in Dg), int(sumD))
    if key in _NC_CACHE:
        return _NC_CACHE[key]

    import concourse.bacc as bacc
    import concourse.bass as bass
    import concourse.tile as tile
    import concourse.mybir as mybir
    from concourse.masks import make_identity

    f32 = mybir.dt.float32
    i32 = mybir.dt.int32
    cumD = np.concatenate([[0], np.cumsum(Dg)]).astype(np.int64)

    nc = bacc.Bacc("TRN2", target_bir_lowering=False, debug=False,
                   num_devices=NCORES)

    xT_t = nc.dram_tensor("xT", [IN_C, NPAD], f32, kind="ExternalInput")
    offs_t = nc.dram_tensor("offs", [P, sumD], i32, kind="ExternalInput")
    dinv_t = nc.dram_tensor("dinv", [P, NGROUP], f32, kind="ExternalInput")
    W1_t = nc.dram_tensor("W1", [IN_C, HID], f32, kind="ExternalInput")
    W2_t = nc.dram_tensor("W2", [HID, OUT_C], f32, kind="ExternalInput")
    b1b_t = nc.dram_tensor("b1b", [P, HID], f32, kind="ExternalInput")
    b2b_t = nc.dram_tensor("b2b", [P, OUT_C], f32, kind="ExternalInput")
    out_t = nc.dram_tensor("out", [NPAD, OUT_C], f32, kind="ExternalOutput")

    hs1_own = nc.dram_tensor("hs1_own", [NPAD, HID], f32)
    hs2_own = nc.dram_tensor("hs2_own", [NPAD, OUT_C], f32)
    table1 = nc.dram_tensor("table1", [VTOT, HID], f32, addr_space="Shared")
    table2 = nc.dram_tensor("table2", [VTOT, OUT_C], f32, addr_space="Shared")

    groups = list(range(NGROUP))
    rg = [list(range(NCORES))]

    with tile.TileContext(nc) as tc:
        with tc.tile_pool(name="const", bufs=1) as cp, \
             tc.tile_pool(name="xt", bufs=4) as xp, \
             tc.tile_pool(name="gat", bufs=2) as gp, \
             tc.tile_pool(name="work", bufs=3) as wp, \
             tc.tile_pool(name="ps1", bufs=2, space="PSUM") as ps1, \
             tc.tile_pool(name="psT", bufs=2, space="PSUM") as psT, \
             tc.tile_pool(name="ps2", bufs=2, space="PSUM") as ps2:

            ident = cp.tile([P, P], f32)
            make_identity(nc, ident[:])
            w1a = cp.tile([P, HID], f32)
            w1b = cp.tile([P, HID], f32)
            nc.sync.dma_start(out=w1a[:], in_=W1_t[0:P, :])
            nc.sync.dma_start(out=w1b[:], in_=W1_t[P:IN_C, :])
            w2s = cp.tile([HID, OUT_C], f32)
            nc.sync.dma_start(out=w2s[:], in_=W2_t[:, :])
            b1s = cp.tile([P, HID], f32)
            nc.sync.dma_start(out=b1s[:], in_=b1b_t[:, :])
            b2s = cp.tile([P, OUT_C], f32)
            nc.sync.dma_start(out=b2s[:], in_=b2b_t[:, :])
            dvs = cp.tile([P, NGROUP], f32)
            nc.sync.dma_start(out=dvs[:], in_=dinv_t[:, :])
            offs_sb = cp.tile([P, sumD], i32)
            nc.sync.dma_start(out=offs_sb[:], in_=offs_t[:, :])

            # ---- P1: hs1_own = dinv * (x W1), per group ----
            for g in groups:
                xt0 = xp.tile([P, P], f32, tag="xt0")
                xt1 = xp.tile([P, P], f32, tag="xt1")
                nc.sync.dma_start(out=xt0[:], in_=xT_t[0:P, g * P:(g + 1) * P])
                nc.sync.dma_start(out=xt1[:], in_=xT_t[P:IN_C, g * P:(g + 1) * P])
                pm = ps1.tile([P, HID], f32)
                nc.tensor.matmul(out=pm[:], lhsT=xt0[:], rhs=w1a[:],
                                 start=True, stop=False)
                nc.tensor.matmul(out=pm[:], lhsT=xt1[:], rhs=w1b[:],
                                 start=False, stop=True)
                hs1 = wp.tile([P, HID], f32, tag="hs1")
                nc.vector.tensor_scalar_mul(hs1[:], pm[:], dvs[:, g:g + 1])
                nc.sync.dma_start(out=hs1_own[g * P:(g + 1) * P, :], in_=hs1[:])

            nc.gpsimd.collective_compute(
                "AllGather", mybir.AluOpType.bypass, replica_groups=rg,
                ins=[hs1_own[:, :]], outs=[table1[:, :]])

            # ---- A1 + L2 projection, per group ----
            for g in groups:
                D = int(Dg[g])
                gt = gp.tile([P, D * HID], f32, tag="g1")
                for j in range(D):
                    col = int(cumD[g]) + j
                    nc.gpsimd.indirect_dma_start(
                        out=gt[:, j * HID:(j + 1) * HID],
                        out_offset=None,
                        in_=table1[:, :],
                        in_offset=bass.IndirectOffsetOnAxis(
                            ap=offs_sb[:, col:col + 1], axis=0),
                    )
                red = wp.tile([P, HID], f32, tag="red1")
                nc.vector.tensor_reduce(
                    out=red[:], in_=gt[:].rearrange("p (d f) -> p f d", f=HID),
                    axis=mybir.AxisListType.X, op=mybir.AluOpType.add)
                u = wp.tile([P, HID], f32, tag="u1")
                nc.vector.tensor_scalar_mul(u[:], red[:], dvs[:, g:g + 1])
                v = wp.tile([P, HID], f32, tag="v1")
                nc.vector.tensor_tensor(out=v[:], in0=u[:], in1=b1s[:],
                                        op=mybir.AluOpType.add)
                h1 = wp.tile([P, HID], f32, tag="h1")
                nc.scalar.activation(out=h1[:], in_=v[:],
                                     func=mybir.ActivationFunctionType.Relu,
                                     scale=dvs[:, g:g + 1])
                # L2 projection: hs2 = (dinv*h1) @ W2  (dinv already folded)
                pT = psT.tile([HID, P], f32)
                nc.tensor.transpose(out=pT[:], in_=h1[:], identity=ident[:])
                h1T = wp.tile([HID, P], f32, tag="h1T")
                nc.vector.tensor_copy(out=h1T[:], in_=pT[:])
                pm2 = ps2.tile([P, OUT_C], f32)
                nc.tensor.matmul(out=pm2[:], lhsT=h1T[:], rhs=w2s[:],
                                 start=True, stop=True)
                hs2 = wp.tile([P, OUT_C], f32, tag="hs2")
                nc.vector.tensor_copy(out=hs2[:], in_=pm2[:])
                nc.sync.dma_start(out=hs2_own[g * P:(g + 1) * P, :], in_=hs2[:])

            nc.gpsimd.collective_compute(
                "AllGather", mybir.AluOpType.bypass, replica_groups=rg,
                ins=[hs2_own[:, :]], outs=[table2[:, :]])

            # ---- A2: final aggregation ----
            for g in groups:
                D = int(Dg[g])
                gt2 = gp.tile([P, D * OUT_C], f32, tag="g2")
                for j in range(D):
                    col = int(cumD[g]) + j
                    nc.gpsimd.indirect_dma_start(
                        out=gt2[:, j * OUT_C:(j + 1) * OUT_C],
                        out_offset=None,
                        in_=table2[:, :],
                        in_offset=bass.IndirectOffsetOnAxis(
                            ap=offs_sb[:, col:col + 1], axis=0),
                    )
                red2 = wp.tile([P, OUT_C], f32, tag="red2")
                nc.vector.tensor_reduce(
                    out=red2[:], in_=gt2[:].rearrange("p (d f) -> p f d", f=OUT_C),
                    axis=mybir.AxisListType.X, op=mybir.AluOpType.add)
                u2 = wp.tile([P, OUT_C], f32, tag="u2")
                nc.vector.tensor_scalar_mul(u2[:], red2[:], dvs[:, g:g + 1])
                o2 = wp.tile([P, OUT_C], f32, tag="o2")
                nc.vector.tensor_tensor(out=o2[:], in0=u2[:], in1=b2s[:],
                                        op=mybir.AluOpType.add)
                nc.sync.dma_start(out=out_t[g * P:(g + 1) * P, :], in_=o2[:])

    nc.compile()
    _NC_CACHE[key] = nc
    return nc


def kernel(x, edge_index, W1, b1, W2, b2):
    from concourse.bass_utils import run_bass_kernel_spmd

    prep = _host_prep(x, edge_index, W1, b1, W2, b2)
    nc = _build_bass(prep["Dg"], prep["sumD"])

    in_maps = []
    for c in range(NCORES):
        in_maps.append({
            "xT": prep["xT"][c],
            "offs": np.ascontiguousarray(prep["offs"][c]),
            "dinv": prep["dinv"][c],
            "W1": prep["W1"],
            "W2": prep["W2"],
            "b1b": prep["b1b"],
            "b2b": prep["b2b"],
        })
    res = run_bass_kernel_spmd(nc, in_maps, core_ids=list(range(NCORES)))

    out = np.empty((N, OUT_C), dtype=np.float32)
    for c in range(NCORES):
        out[prep["perms"][c]] = res.results[c]["out"][:NPC]
    return out


if __name__ == "__main__":
    rng = np.random.default_rng(0)
    x = rng.standard_normal((N, IN_C)).astype(np.float32)
    ei = rng.integers(0, N, size=(2, 3200000)).astype(np.int64)
    W1 = rng.standard_normal((IN_C, HID)).astype(np.float32) / 16.0
    W2 = rng.standard_normal((HID, OUT_C)).astype(np.float32) / 5.66
    out = kernel(x, ei, W1, np.zeros(HID, np.float32), W2,
                 np.zeros(OUT_C, np.float32))
    print(out.shape, out.dtype, np.abs(out).mean())
